# revision 53
# baseline (speedup 1.0000x reference)
"""Causal self-attention (B=2, T=2048, C=1024, H=16) on 8 TRN2 NeuronCores.

Sharding: core c handles batch b = c//4 and heads 4*(c%4) .. 4*(c%4)+3
(data-parallel over B, tensor-parallel over heads; full K/V for its heads
is computed locally from the core's QKV projection slice).

Per-core dataflow (all matmuls bf16 inputs, fp32 PSUM accumulation):
  - host passes xT = x[b].T [C,T], wqk = [Wq_h|Wk_h]^T [C,512],
    wv = [Wv_h0|0|...|Wv_h3|0]^T [C,260] (65-wide blocks, last col zero),
    and a [128,128] causal-mask tile, all bf16
  - qT [128,T] per pair via projection matmuls (contraction c on partitions)
  - kT per head is stored ZERO-PADDED to 128 partitions: kpe = [k_even; 0],
    kpo = [0; k_odd].  S matmuls then run with K=128 in the SAME 128x128 PE
    array configuration as every other matmul in the kernel -- one mode for
    the whole PE stream means no 64<->128 reconfiguration drains and
    back-to-back matmuls pipeline at N cycles.  (The pad rows multiply the
    other head's q rows by zero.)
  - v [t,260] with a ones column appended per head (65th of each block)
  - s [k=128, 1024] = [S_even|S_odd] per chunk; exp on ScalarE (scale=1/8
    fused) over only the causally-live columns; causal masking of the
    partial diagonal 128-col block is a DVE multiply with the precomputed
    mask (GpSimd's in-order queue is reserved for the normalize
    partition-broadcasts -- a gpsimd mask would queue behind exps and stall
    the normalize chain, which the next block's PV allocation waits on)
  - y^T [65, 512] += V'.T @ P^T accumulated over k-tiles; row 64 = softmax
    denominators (from the ones column)
  - normalize per q-block: the PSUM-freeing copies run first (so the next
    block's ye/yo never waits), then reciprocal_approx_fast + gpsimd
    partition_broadcast + multiply; DMA y^T[h] [64,512] out per block
  - host transposes/concats heads.

Schedule: one flat software pipeline over both pairs, block-interleaved
(p0,b0),(p1,b0),(p0,b1),... with PV lagging S by 4 chunks so ScalarE's exp
queue stays full; projection chains are front-loaded into the preceding
sub-block's first chunks (their CASTs must clear the DVE queue before the
block-boundary normalize lands on it).  PSUM: 2 double-bank s slots +
2 chain banks + ye/yo = 8 banks exactly.
"""

import os
import sys
import types
import numpy as np
import ml_dtypes

import concourse.bass as bass
import concourse.mybir as mybir
import concourse.tile as tile
from concourse import bacc
from concourse.bass_utils import run_bass_kernel_spmd

B, T, C, H = 2, 2048, 1024, 16
D = 64
NCORES = 8
HPC = 4          # heads per core
NQB = 4          # q blocks of 512
QB = 512
F32 = mybir.dt.float32
F32R = mybir.dt.float32r
BF16 = mybir.dt.bfloat16
NPBF16 = ml_dtypes.bfloat16
EXP = mybir.ActivationFunctionType.Exp
MULT = mybir.AluOpType.mult
IS_GE = mybir.AluOpType.is_ge


def _install_profhook():
    """Register the NTFF profile hook shim so BASS_TRACE=1 works; harmless
    no-op (graceful trace skip) when the axon .so lacks profiling."""
    if "antenv.axon_hooks" not in sys.modules:
        mod = types.ModuleType("antenv.axon_hooks")
        mod._hook = None
        mod.set_axon_ntff_profile_hook = lambda h: setattr(mod, "_hook", h)
        mod.get_axon_ntff_profile_hook = lambda: mod._hook
        sys.modules["antenv.axon_hooks"] = mod
        try:
            import antenv
            antenv.axon_hooks = mod
        except ImportError:
            pass
    try:
        from trn_agent_boot.trn_boot import _ntff_profile_via_ctypes
        sys.modules["antenv.axon_hooks"].set_axon_ntff_profile_hook(
            _ntff_profile_via_ctypes("/opt/axon/libaxon_pjrt.so")
        )
        import concourse.bass_utils as bu
        bu.upload_artifacts = lambda tmpdir: tmpdir
    except Exception:
        pass


_install_profhook()

_NC = None


def _build():
    nc = bacc.Bacc("TRN2", target_bir_lowering=False, debug=False,
                   num_devices=NCORES)
    xT_d = nc.declare_dram_parameter("xT", [C, T], BF16, isOutput=False)
    wqk_d = nc.declare_dram_parameter("wqk", [C, 2 * HPC * D], BF16,
                                      isOutput=False)
    wv_d = nc.declare_dram_parameter("wv", [C, HPC * 65], BF16,
                                     isOutput=False)
    cm_d = nc.declare_dram_parameter("cm", [128, 128], BF16, isOutput=False)
    y_d = nc.declare_dram_parameter("y", [HPC, D, T], F32, isOutput=True)

    from contextlib import ExitStack
    with tile.TileContext(nc) as tc, ExitStack() as ctx:
        sb = ctx.enter_context(tc.tile_pool(name="sb", bufs=1))
        pp = ctx.enter_context(tc.tile_pool(name="pp", bufs=10))
        yp = ctx.enter_context(tc.tile_pool(name="yp", bufs=4))
        # one shared PSUM pool of 3 double-bank slots for s tiles AND
        # projection chains (6 banks) + ye/yo (2 banks): the 3-deep window
        # keeps ScalarE's exp queue full; chain CASTs clear the DVE queue
        # promptly now that masks are small and the normalize is reordered
        ps = ctx.enter_context(tc.tile_pool(name="ps", bufs=3, space="PSUM"))
        psy = ctx.enter_context(tc.tile_pool(name="psy", bufs=1, space="PSUM"))

        # per-(c, t-block) x^T tiles; per-t-block q/k tiles -> fine-grain deps
        xTt = [[sb.tile([128, 512], BF16, name=f"xT{c}_{tb}")
                for tb in range(4)] for c in range(8)]
        wqks = [sb.tile([128, 512], BF16, name=f"wqk{i}") for i in range(8)]
        wvs = [sb.tile([128, 260], BF16, name=f"wv{i}") for i in range(8)]
        qs = [[sb.tile([128, 512], BF16, name=f"q{p}_{tb}") for tb in range(4)]
              for p in range(2)]
        # zero-padded per-head k tiles: kpe = [k_even; 0], kpo = [0; k_odd]
        kpe = [[sb.tile([128, 512], BF16, name=f"kpe{p}_{tb}")
                for tb in range(4)] for p in range(2)]
        kpo = [[sb.tile([128, 512], BF16, name=f"kpo{p}_{tb}")
                for tb in range(4)] for p in range(2)]
        vs = [sb.tile([128, 260], BF16, name=f"v_{t}") for t in range(16)]
        ones2 = sb.tile([128, 4], F32, name="ones2")

        # warm-up first: keep the PE's HAM activity monitor busy while the
        # input DMAs land, so real matmuls start at 2.4 GHz instead of 1.2
        wupf = sb.tile([128, 512], F32, name="wupf")
        nc.gpsimd.memset(wupf[:], 0.5)
        nc.gpsimd.memset(ones2[:], 1.0)
        wup = sb.tile([128, 512], F32R, name="wup")
        nc.vector.tensor_copy(wup[:], wupf[:])
        wups = ps.tile([128, 512], F32, name="wups", tag="s")
        for _ in range(8):
            nc.tensor.matmul(wups[:], wup[:, 0:128], wup[:], start=True,
                             stop=True)
        # zero the pad halves of the k tiles on gpsimd (idle at start;
        # emitted after the warmup memsets so the PE warmup isn't delayed,
        # and off the DVE queue so early projection CASTs aren't)
        for p in range(2):
            for tb in range(4):
                nc.gpsimd.memset(kpe[p][tb][64:128, :], 0.0)
                nc.gpsimd.memset(kpo[p][tb][0:64, :], 0.0)

        # host-provided causal-mask tile (keep iff col >= row); the per-chunk
        # mask is a cheap DVE multiply, keeping GpSimd's in-order queue free
        # for the normalize partition-broadcasts (a gpsimd mask queues behind
        # exps and would delay them).
        cmt = sb.tile([128, 128], BF16, name="cmt")
        nc.sync.dma_start(cmt[:], cm_d.ap()[:, :])

        # DMA order: (wqk[c], xT[c,0]) pairs so the first projection chain
        # can start after ~2 transfers; then wv, then xT t-blocks 1..3
        for c in range(8):
            cr = slice(c * 128, (c + 1) * 128)
            nc.sync.dma_start(wqks[c][:], wqk_d.ap()[cr, :])
            nc.sync.dma_start(xTt[c][0][:], xT_d.ap()[cr, 0:512])
        for c in range(8):
            nc.sync.dma_start(wvs[c][:], wv_d.ap()[c * 128:(c + 1) * 128, :])
        for tb in range(1, 4):
            for c in range(8):
                nc.sync.dma_start(
                    xTt[c][tb][:],
                    xT_d.ap()[c * 128:(c + 1) * 128, tb * 512:(tb + 1) * 512])

        def qk_chain(p, ft_kind, tb):
            """One projection chain: q (ft_kind=0) or k (ft_kind=1) of pair p,
            t-block tb.  q lands in qs; k is split into the two zero-padded
            per-head tiles (no partition shift: even head keeps rows 0:64,
            odd head keeps rows 64:128)."""
            ft = p if ft_kind == 0 else 2 + p
            mm = ps.tile([128, 512], F32, name=f"pqk{p}_{ft}_{tb}", tag="s")
            for c in range(8):
                nc.tensor.matmul(mm[:],
                                 wqks[c][:, ft * 128:(ft + 1) * 128],
                                 xTt[c][tb][:],
                                 start=(c == 0), stop=(c == 7))
            if ft_kind == 0:
                nc.vector.tensor_copy(qs[p][tb][:], mm[:])
            else:
                nc.vector.tensor_copy(kpe[p][tb][0:64, :], mm[0:64, :])
                nc.vector.tensor_copy(kpo[p][tb][64:128, :], mm[64:128, :])

        def v_chain(tt):
            """Combined v projection for one t-tile (all 4 heads, N=260)."""
            tb, sub = tt // 4, tt % 4
            mmv = ps.tile([128, 260], F32, name=f"pv{tt}", tag="s")
            for c in range(8):
                nc.tensor.matmul(mmv[:],
                                 xTt[c][tb][:, sub * 128:(sub + 1) * 128],
                                 wvs[c][:],
                                 start=(c == 0), stop=(c == 7))
            nc.vector.tensor_copy(vs[tt][:], mmv[:])
            nc.vector.tensor_copy(vs[tt][:, 64:260:65], ones2[:])

        ptiles = {}

        def attn_s_part(p, j, kk):
            """S matmuls + exp + causal mask for chunk (p, j, kk).

            Diagonal k-tiles only have valid scores for q >= k, i.e. local
            q >= off = 128*(kk-4j); the matmuls/exp/PV all start at column
            off, and the mask multiply zeroes the partial 128-col block's
            lower triangle."""
            off = max(0, 128 * (kk - 4 * j))
            roff = off
            s = ps.tile([128, 1024], F32, name=f"s{p}_{j}_{kk}", tag="s")
            ktb, ksub = kk // 4, (kk % 4) * 128
            nc.tensor.matmul(s[:, roff:512],
                             kpe[p][ktb][:, ksub:ksub + 128],
                             qs[p][j][:, roff:512],
                             start=True, stop=True)
            nc.tensor.matmul(s[:, 512 + roff:1024],
                             kpo[p][ktb][:, ksub:ksub + 128],
                             qs[p][j][:, roff:512],
                             start=True, stop=True)
            pt = pp.tile([128, 1024], BF16, name=f"pt{p}_{j}_{kk}", tag="pt")
            sv = s[:].rearrange("p (b q) -> p b q", b=2)[:, :, roff:512]
            pv = pt[:].rearrange("p (b q) -> p b q", b=2)[:, :, roff:512]
            nc.scalar.activation(pv, sv, EXP, scale=0.125)
            if kk >= 4 * j:
                # causal mask both head halves in one DVE multiply over just
                # the partial 128-col diagonal block at [off, off+128)
                v3 = pt[:].rearrange("p (b q) -> p b q", b=2)[:, :,
                                                             off:off + 128]
                m3 = cmt[:].rearrange("p (b q) -> p b q", b=1).to_broadcast(
                    (128, 2, 128))
                nc.vector.tensor_tensor(v3, v3, m3, op=MULT)
            ptiles[(p, j, kk)] = pt

        states = {}

        def normalize(p, j, state):
            """Both heads fused: the four PSUM-freeing copies run FIRST (so
            the next block's ye/yo allocation never waits on the rest of the
            chain), then one stacked recip/broadcast/multiply and one DMA."""
            ye, yo = state["ye"], state["yo"]
            # for the very last block ScalarE is idle: run the odd head's
            # copies there so both heads' normalize chains overlap in the
            # kernel's tail
            last_blk = (p, j) == (1, NQB - 1)
            ysbe = yp.tile([65, 512], F32, name=f"ysbe{p}_{j}", tag="ysbe")
            nc.vector.tensor_copy(ysbe[:], ye[:])
            ysbo = yp.tile([65, 512], F32, name=f"ysbo{p}_{j}", tag="ysbo")
            (nc.scalar.copy if last_blk else nc.vector.tensor_copy)(
                ysbo[:], yo[:])
            sse = yp.tile([1, 512], F32, name=f"sse{p}_{j}", tag="sse")
            nc.vector.tensor_copy(sse[:], ye[64:65, :])
            sso = yp.tile([1, 512], F32, name=f"sso{p}_{j}", tag="sso")
            (nc.scalar.copy if last_blk else nc.vector.tensor_copy)(
                sso[:], yo[64:65, :])
            rse = yp.tile([1, 512], F32, name=f"rse{p}_{j}", tag="rse")
            nc.vector.reciprocal_approx_fast(out=rse[:], in_=sse[:])
            rso = yp.tile([1, 512], F32, name=f"rso{p}_{j}", tag="rso")
            nc.vector.reciprocal_approx_fast(out=rso[:], in_=sso[:])
            bse = yp.tile([64, 512], F32, name=f"bse{p}_{j}", tag="bse")
            nc.gpsimd.partition_broadcast(bse[:], rse[:], channels=64)
            bso = yp.tile([64, 512], F32, name=f"bso{p}_{j}", tag="bso")
            nc.gpsimd.partition_broadcast(bso[:], rso[:], channels=64)
            yne = yp.tile([64, 512], F32, name=f"yne{p}_{j}", tag="yne")
            nc.vector.tensor_tensor(yne[:], ysbe[0:64, :], bse[:], op=MULT)
            nc.sync.dma_start(
                y_d.ap()[2 * p, :, j * 512:(j + 1) * 512], yne[:])
            yno = yp.tile([64, 512], F32, name=f"yno{p}_{j}", tag="yno")
            nc.vector.tensor_tensor(yno[:], ysbo[0:64, :], bso[:], op=MULT)
            nc.sync.dma_start(
                y_d.ap()[2 * p + 1, :, j * 512:(j + 1) * 512], yno[:])

        def attn_pv_part(p, j, kk):
            """PV accumulation for chunk (p, j, kk); finishes the q-block
            with normalize on its last k-tile."""
            state = states.setdefault((p, j), {})
            nkt = 4 * (j + 1)
            if kk == 0:
                state["ye"] = psy.tile([65, 512], F32,
                                       name=f"ye{p}_{j}", tag="ye")
                state["yo"] = psy.tile([65, 512], F32,
                                       name=f"yo{p}_{j}", tag="yo")
            pt = ptiles.pop((p, j, kk))
            first, last = (kk == 0), (kk == nkt - 1)
            # skip columns left of off (all-zero P above the causal
            # diagonal); their y contribution is zero and PSUM keeps the
            # prior partials there
            roff = 0 if first else max(0, 128 * (kk - 4 * j))
            nc.tensor.matmul(state["ye"][:, roff:512],
                             vs[kk][:, 130 * p:130 * p + 65],
                             pt[:, roff:512],
                             start=first, stop=last)
            nc.tensor.matmul(state["yo"][:, roff:512],
                             vs[kk][:, 130 * p + 65:130 * p + 130],
                             pt[:, 512 + roff:1024],
                             start=first, stop=last)
            if last:
                normalize(p, j, state)

        # One flat software pipeline over BOTH pairs, block-interleaved:
        # (p0,b0),(p1,b0),(p0,b1),(p1,b1),...  PV lags S by 3 chunks so
        # ScalarE's exp queue stays full; projection chains are spread across
        # the PRECEDING sub-block's chunks as PE filler (work for (p1,t)
        # during (p0,t); work for (p0,t+1) and its v tiles during (p1,t)).
        seq = [(p, t, kk) for t in range(NQB) for p in range(2)
               for kk in range(4 * (t + 1))]
        work_during = {}
        for t in range(NQB):
            work_during[(0, t)] = [lambda t=t: qk_chain(1, 0, t),
                                   lambda t=t: qk_chain(1, 1, t)]
            if t < NQB - 1:
                work_during[(1, t)] = (
                    [lambda t=t: qk_chain(0, 0, t + 1),
                     lambda t=t: qk_chain(0, 1, t + 1)] +
                    [lambda tt=tt: v_chain(tt)
                     for tt in range(4 * (t + 1), 4 * (t + 1) + 4)])
        # upfront: pair-0 stage 0 + its v tiles
        qk_chain(0, 0, 0)
        qk_chain(0, 1, 0)
        for tt in range(4):
            v_chain(tt)

        pend = []
        queue = []
        for (p, t, kk) in seq:
            blk = 4 * (t + 1)
            if kk == 0:
                queue = list(work_during.get((p, t), ()))
            attn_s_part(p, t, kk)
            # front-load filler: the CASTs must complete well before the
            # block boundary, where the previous normalize clogs the DVE
            for _ in range(min(2, len(queue))):
                queue.pop(0)()
            pend.append((p, t, kk))
            if len(pend) > 4:
                pp_, pj, pkk = pend.pop(0)
                attn_pv_part(pp_, pj, pkk)
        for pp_, pj, pkk in pend:
            attn_pv_part(pp_, pj, pkk)

    nc.compile()
    return nc


def _get_nc():
    global _NC
    if _NC is None:
        _NC = _build()
    return _NC


def _make_in_maps(x, W_attn):
    x = np.asarray(x, dtype=np.float32)
    W = np.asarray(W_attn, dtype=np.float32)
    wq, wk, wv = W[0:C], W[C:2 * C], W[2 * C:3 * C]
    in_maps = []
    for c in range(NCORES):
        b, g = c // 4, c % 4
        heads = [HPC * g + i for i in range(HPC)]
        xTb = np.ascontiguousarray(x[b].T).astype(NPBF16)
        qrows = np.concatenate([wq[D * h:D * h + D] for h in heads], axis=0)
        krows = np.concatenate([wk[D * h:D * h + D] for h in heads], axis=0)
        wqk_np = np.ascontiguousarray(
            np.concatenate([qrows, krows], 0).T).astype(NPBF16)
        wv_np = np.zeros((C, HPC * 65), np.float32)
        for i, h in enumerate(heads):
            wv_np[:, 65 * i:65 * i + D] = wv[D * h:D * h + D].T
        in_maps.append({"xT": xTb, "wqk": wqk_np, "wv": wv_np.astype(NPBF16),
                        "cm": _causal_masks()})
    return in_maps


def _causal_masks():
    r = np.arange(128)[:, None]
    return (np.arange(128)[None, :] >= r).astype(NPBF16)


def _execute(in_maps, trace=False):
    return run_bass_kernel_spmd(_get_nc(), in_maps,
                                core_ids=list(range(NCORES)), trace=trace)


def _assemble(results):
    y = np.empty((B, T, C), np.float32)
    for c in range(NCORES):
        b, g = c // 4, c % 4
        yc = results[c]["y"]
        for i in range(HPC):
            h = HPC * g + i
            y[b, :, D * h:D * h + D] = yc[i].T
    return y


def kernel(x, W_attn):
    res = _execute(_make_in_maps(x, W_attn), trace=False)
    return _assemble(res.results)


# revision 54
# speedup vs baseline: 1.0086x; 1.0086x over previous
"""Causal self-attention (B=2, T=2048, C=1024, H=16) on 8 TRN2 NeuronCores.

Sharding: core c handles batch b = c//4 and heads 4*(c%4) .. 4*(c%4)+3
(data-parallel over B, tensor-parallel over heads; full K/V for its heads
is computed locally from the core's QKV projection slice).

Per-core dataflow (all matmuls bf16 inputs, fp32 PSUM accumulation):
  - host passes xT = x[b].T [C,T], wqk = [Wq_h|Wk_h]^T [C,512],
    wv = [Wv_h0|0|...|Wv_h3|0]^T [C,260] (65-wide blocks, last col zero),
    and a [128,128] causal-mask tile, all bf16
  - qT [128,T] per pair via projection matmuls (contraction c on partitions)
  - kT per head is stored ZERO-PADDED to 128 partitions: kpe = [k_even; 0],
    kpo = [0; k_odd].  S matmuls then run with K=128 in the SAME 128x128 PE
    array configuration as every other matmul in the kernel -- one mode for
    the whole PE stream means no 64<->128 reconfiguration drains and
    back-to-back matmuls pipeline at N cycles.  (The pad rows multiply the
    other head's q rows by zero.)
  - v [t,260] with a ones column appended per head (65th of each block)
  - s [k=128, 1024] = [S_even|S_odd] per chunk; exp on ScalarE (scale=1/8
    fused) over only the causally-live columns; causal masking of the
    partial diagonal 128-col block is a DVE multiply with the precomputed
    mask (GpSimd's in-order queue is reserved for the normalize
    partition-broadcasts -- a gpsimd mask would queue behind exps and stall
    the normalize chain, which the next block's PV allocation waits on)
  - y^T [65, 512] += V'.T @ P^T accumulated over k-tiles; row 64 = softmax
    denominators (from the ones column)
  - normalize per q-block: the PSUM-freeing copies run first (so the next
    block's ye/yo never waits), then reciprocal_approx_fast + gpsimd
    partition_broadcast + multiply; DMA y^T[h] [64,512] out per block
  - host transposes/concats heads.

Schedule: one flat software pipeline over both pairs, block-interleaved
(p0,b0),(p1,b0),(p0,b1),... with PV lagging S by 4 chunks so ScalarE's exp
queue stays full; projection chains are front-loaded into the preceding
sub-block's first chunks (their CASTs must clear the DVE queue before the
block-boundary normalize lands on it).  PSUM: 2 double-bank s slots +
2 chain banks + ye/yo = 8 banks exactly.
"""

import os
import sys
import types
import numpy as np
import ml_dtypes

import concourse.bass as bass
import concourse.mybir as mybir
import concourse.tile as tile
from concourse import bacc
from concourse.bass_utils import run_bass_kernel_spmd

B, T, C, H = 2, 2048, 1024, 16
D = 64
NCORES = 8
HPC = 4          # heads per core
NQB = 4          # q blocks of 512
QB = 512
F32 = mybir.dt.float32
F32R = mybir.dt.float32r
BF16 = mybir.dt.bfloat16
NPBF16 = ml_dtypes.bfloat16
EXP = mybir.ActivationFunctionType.Exp
MULT = mybir.AluOpType.mult
IS_GE = mybir.AluOpType.is_ge


def _install_profhook():
    """Register the NTFF profile hook shim so BASS_TRACE=1 works; harmless
    no-op (graceful trace skip) when the axon .so lacks profiling."""
    if "antenv.axon_hooks" not in sys.modules:
        mod = types.ModuleType("antenv.axon_hooks")
        mod._hook = None
        mod.set_axon_ntff_profile_hook = lambda h: setattr(mod, "_hook", h)
        mod.get_axon_ntff_profile_hook = lambda: mod._hook
        sys.modules["antenv.axon_hooks"] = mod
        try:
            import antenv
            antenv.axon_hooks = mod
        except ImportError:
            pass
    try:
        from trn_agent_boot.trn_boot import _ntff_profile_via_ctypes
        sys.modules["antenv.axon_hooks"].set_axon_ntff_profile_hook(
            _ntff_profile_via_ctypes("/opt/axon/libaxon_pjrt.so")
        )
        import concourse.bass_utils as bu
        bu.upload_artifacts = lambda tmpdir: tmpdir
    except Exception:
        pass


_install_profhook()

_NC = None


def _build():
    nc = bacc.Bacc("TRN2", target_bir_lowering=False, debug=False,
                   num_devices=NCORES)
    xT_d = nc.declare_dram_parameter("xT", [C, T], BF16, isOutput=False)
    wqk_d = nc.declare_dram_parameter("wqk", [C, 2 * HPC * D], BF16,
                                      isOutput=False)
    wv_d = nc.declare_dram_parameter("wv", [C, HPC * 65], BF16,
                                     isOutput=False)
    cm_d = nc.declare_dram_parameter("cm", [128, 128], BF16, isOutput=False)
    y_d = nc.declare_dram_parameter("y", [HPC, D, T], F32, isOutput=True)

    from contextlib import ExitStack
    with tile.TileContext(nc) as tc, ExitStack() as ctx:
        sb = ctx.enter_context(tc.tile_pool(name="sb", bufs=1))
        pp = ctx.enter_context(tc.tile_pool(name="pp", bufs=10))
        yp = ctx.enter_context(tc.tile_pool(name="yp", bufs=4))
        # dedicated pools: chains double-buffered (2 banks) so S matmuls
        # never wait on a chain's CAST, s tiles 2x2 banks, ye/yo 2 banks
        psp = ctx.enter_context(tc.tile_pool(name="psp", bufs=2, space="PSUM"))
        ps = ctx.enter_context(tc.tile_pool(name="ps", bufs=2, space="PSUM"))
        psy = ctx.enter_context(tc.tile_pool(name="psy", bufs=1, space="PSUM"))

        # per-(c, t-block) x^T tiles; per-t-block q/k tiles -> fine-grain deps
        xTt = [[sb.tile([128, 512], BF16, name=f"xT{c}_{tb}")
                for tb in range(4)] for c in range(8)]
        wqks = [sb.tile([128, 512], BF16, name=f"wqk{i}") for i in range(8)]
        wvs = [sb.tile([128, 260], BF16, name=f"wv{i}") for i in range(8)]
        qs = [[sb.tile([128, 512], BF16, name=f"q{p}_{tb}") for tb in range(4)]
              for p in range(2)]
        # zero-padded per-head k tiles: kpe = [k_even; 0], kpo = [0; k_odd]
        kpe = [[sb.tile([128, 512], BF16, name=f"kpe{p}_{tb}")
                for tb in range(4)] for p in range(2)]
        kpo = [[sb.tile([128, 512], BF16, name=f"kpo{p}_{tb}")
                for tb in range(4)] for p in range(2)]
        vs = [sb.tile([128, 260], BF16, name=f"v_{t}") for t in range(16)]
        ones2 = sb.tile([128, 4], F32, name="ones2")

        # warm-up first: keep the PE's HAM activity monitor busy while the
        # input DMAs land, so real matmuls start at 2.4 GHz instead of 1.2
        wupf = sb.tile([128, 512], F32, name="wupf")
        nc.gpsimd.memset(wupf[:], 0.5)
        nc.gpsimd.memset(ones2[:], 1.0)
        wup = sb.tile([128, 512], F32R, name="wup")
        nc.vector.tensor_copy(wup[:], wupf[:])
        wups = psp.tile([128, 512], F32, name="wups", tag="pmm")
        for _ in range(8):
            nc.tensor.matmul(wups[:], wup[:, 0:128], wup[:], start=True,
                             stop=True)
        # zero the pad halves of the k tiles on gpsimd (idle at start;
        # emitted after the warmup memsets so the PE warmup isn't delayed,
        # and off the DVE queue so early projection CASTs aren't)
        for p in range(2):
            for tb in range(4):
                nc.gpsimd.memset(kpe[p][tb][64:128, :], 0.0)
                nc.gpsimd.memset(kpo[p][tb][0:64, :], 0.0)

        # host-provided causal-mask tile (keep iff col >= row); the per-chunk
        # mask is a cheap DVE multiply, keeping GpSimd's in-order queue free
        # for the normalize partition-broadcasts (a gpsimd mask queues behind
        # exps and would delay them).
        cmt = sb.tile([128, 128], BF16, name="cmt")
        nc.sync.dma_start(cmt[:], cm_d.ap()[:, :])

        # DMA order: (wqk[c], xT[c,0]) pairs so the first projection chain
        # can start after ~2 transfers; then wv, then xT t-blocks 1..3
        for c in range(8):
            cr = slice(c * 128, (c + 1) * 128)
            nc.sync.dma_start(wqks[c][:], wqk_d.ap()[cr, :])
            nc.sync.dma_start(xTt[c][0][:], xT_d.ap()[cr, 0:512])
        for c in range(8):
            nc.sync.dma_start(wvs[c][:], wv_d.ap()[c * 128:(c + 1) * 128, :])
        for tb in range(1, 4):
            for c in range(8):
                nc.sync.dma_start(
                    xTt[c][tb][:],
                    xT_d.ap()[c * 128:(c + 1) * 128, tb * 512:(tb + 1) * 512])

        def qk_chain(p, ft_kind, tb):
            """One projection chain: q (ft_kind=0) or k (ft_kind=1) of pair p,
            t-block tb.  q lands in qs; k is split into the two zero-padded
            per-head tiles (no partition shift: even head keeps rows 0:64,
            odd head keeps rows 64:128)."""
            ft = p if ft_kind == 0 else 2 + p
            mm = psp.tile([128, 512], F32, name=f"pqk{p}_{ft}_{tb}", tag="pmm")
            for c in range(8):
                nc.tensor.matmul(mm[:],
                                 wqks[c][:, ft * 128:(ft + 1) * 128],
                                 xTt[c][tb][:],
                                 start=(c == 0), stop=(c == 7))
            if ft_kind == 0:
                nc.vector.tensor_copy(qs[p][tb][:], mm[:])
            else:
                nc.vector.tensor_copy(kpe[p][tb][0:64, :], mm[0:64, :])
                nc.vector.tensor_copy(kpo[p][tb][64:128, :], mm[64:128, :])

        def v_chain(tt):
            """Combined v projection for one t-tile (all 4 heads, N=260)."""
            tb, sub = tt // 4, tt % 4
            mmv = psp.tile([128, 260], F32, name=f"pv{tt}", tag="pmm")
            for c in range(8):
                nc.tensor.matmul(mmv[:],
                                 xTt[c][tb][:, sub * 128:(sub + 1) * 128],
                                 wvs[c][:],
                                 start=(c == 0), stop=(c == 7))
            nc.vector.tensor_copy(vs[tt][:], mmv[:])
            nc.vector.tensor_copy(vs[tt][:, 64:260:65], ones2[:])

        ptiles = {}

        def attn_s_part(p, j, kk):
            """S matmuls + exp + causal mask for chunk (p, j, kk).

            Diagonal k-tiles only have valid scores for q >= k, i.e. local
            q >= off = 128*(kk-4j); the matmuls/exp/PV all start at column
            off, and the mask multiply zeroes the partial 128-col block's
            lower triangle."""
            off = max(0, 128 * (kk - 4 * j))
            roff = off
            s = ps.tile([128, 1024], F32, name=f"s{p}_{j}_{kk}", tag="s")
            ktb, ksub = kk // 4, (kk % 4) * 128
            nc.tensor.matmul(s[:, roff:512],
                             kpe[p][ktb][:, ksub:ksub + 128],
                             qs[p][j][:, roff:512],
                             start=True, stop=True)
            nc.tensor.matmul(s[:, 512 + roff:1024],
                             kpo[p][ktb][:, ksub:ksub + 128],
                             qs[p][j][:, roff:512],
                             start=True, stop=True)
            pt = pp.tile([128, 1024], BF16, name=f"pt{p}_{j}_{kk}", tag="pt")
            sv = s[:].rearrange("p (b q) -> p b q", b=2)[:, :, roff:512]
            pv = pt[:].rearrange("p (b q) -> p b q", b=2)[:, :, roff:512]
            nc.scalar.activation(pv, sv, EXP, scale=0.125)
            if kk >= 4 * j:
                # causal mask both head halves in one DVE multiply over just
                # the partial 128-col diagonal block at [off, off+128)
                v3 = pt[:].rearrange("p (b q) -> p b q", b=2)[:, :,
                                                             off:off + 128]
                m3 = cmt[:].rearrange("p (b q) -> p b q", b=1).to_broadcast(
                    (128, 2, 128))
                nc.vector.tensor_tensor(v3, v3, m3, op=MULT)
            ptiles[(p, j, kk)] = pt

        states = {}

        def normalize(p, j, state):
            """Both heads fused: the four PSUM-freeing copies run FIRST (so
            the next block's ye/yo allocation never waits on the rest of the
            chain), then one stacked recip/broadcast/multiply and one DMA."""
            ye, yo = state["ye"], state["yo"]
            # for the very last block ScalarE is idle: run the odd head's
            # copies there so both heads' normalize chains overlap in the
            # kernel's tail
            last_blk = (p, j) == (1, NQB - 1)
            ysbe = yp.tile([65, 512], F32, name=f"ysbe{p}_{j}", tag="ysbe")
            nc.vector.tensor_copy(ysbe[:], ye[:])
            ysbo = yp.tile([65, 512], F32, name=f"ysbo{p}_{j}", tag="ysbo")
            (nc.scalar.copy if last_blk else nc.vector.tensor_copy)(
                ysbo[:], yo[:])
            sse = yp.tile([1, 512], F32, name=f"sse{p}_{j}", tag="sse")
            nc.vector.tensor_copy(sse[:], ye[64:65, :])
            sso = yp.tile([1, 512], F32, name=f"sso{p}_{j}", tag="sso")
            (nc.scalar.copy if last_blk else nc.vector.tensor_copy)(
                sso[:], yo[64:65, :])
            rse = yp.tile([1, 512], F32, name=f"rse{p}_{j}", tag="rse")
            nc.vector.reciprocal_approx_fast(out=rse[:], in_=sse[:])
            rso = yp.tile([1, 512], F32, name=f"rso{p}_{j}", tag="rso")
            nc.vector.reciprocal_approx_fast(out=rso[:], in_=sso[:])
            bse = yp.tile([64, 512], F32, name=f"bse{p}_{j}", tag="bse")
            nc.gpsimd.partition_broadcast(bse[:], rse[:], channels=64)
            bso = yp.tile([64, 512], F32, name=f"bso{p}_{j}", tag="bso")
            nc.gpsimd.partition_broadcast(bso[:], rso[:], channels=64)
            yne = yp.tile([64, 512], F32, name=f"yne{p}_{j}", tag="yne")
            nc.vector.tensor_tensor(yne[:], ysbe[0:64, :], bse[:], op=MULT)
            nc.sync.dma_start(
                y_d.ap()[2 * p, :, j * 512:(j + 1) * 512], yne[:])
            yno = yp.tile([64, 512], F32, name=f"yno{p}_{j}", tag="yno")
            nc.vector.tensor_tensor(yno[:], ysbo[0:64, :], bso[:], op=MULT)
            nc.sync.dma_start(
                y_d.ap()[2 * p + 1, :, j * 512:(j + 1) * 512], yno[:])

        def attn_pv_part(p, j, kk):
            """PV accumulation for chunk (p, j, kk); finishes the q-block
            with normalize on its last k-tile."""
            state = states.setdefault((p, j), {})
            nkt = 4 * (j + 1)
            if kk == 0:
                state["ye"] = psy.tile([65, 512], F32,
                                       name=f"ye{p}_{j}", tag="ye")
                state["yo"] = psy.tile([65, 512], F32,
                                       name=f"yo{p}_{j}", tag="yo")
            pt = ptiles.pop((p, j, kk))
            first, last = (kk == 0), (kk == nkt - 1)
            # skip columns left of off (all-zero P above the causal
            # diagonal); their y contribution is zero and PSUM keeps the
            # prior partials there
            roff = 0 if first else max(0, 128 * (kk - 4 * j))
            nc.tensor.matmul(state["ye"][:, roff:512],
                             vs[kk][:, 130 * p:130 * p + 65],
                             pt[:, roff:512],
                             start=first, stop=last)
            nc.tensor.matmul(state["yo"][:, roff:512],
                             vs[kk][:, 130 * p + 65:130 * p + 130],
                             pt[:, 512 + roff:1024],
                             start=first, stop=last)
            if last:
                normalize(p, j, state)

        # One flat software pipeline over BOTH pairs, block-interleaved:
        # (p0,b0),(p1,b0),(p0,b1),(p1,b1),...  PV lags S by 3 chunks so
        # ScalarE's exp queue stays full; projection chains are spread across
        # the PRECEDING sub-block's chunks as PE filler (work for (p1,t)
        # during (p0,t); work for (p0,t+1) and its v tiles during (p1,t)).
        seq = [(p, t, kk) for t in range(NQB) for p in range(2)
               for kk in range(4 * (t + 1))]
        work_during = {}
        for t in range(NQB):
            work_during[(0, t)] = [lambda t=t: qk_chain(1, 0, t),
                                   lambda t=t: qk_chain(1, 1, t)]
            if t < NQB - 1:
                work_during[(1, t)] = (
                    [lambda t=t: qk_chain(0, 0, t + 1),
                     lambda t=t: qk_chain(0, 1, t + 1)] +
                    [lambda tt=tt: v_chain(tt)
                     for tt in range(4 * (t + 1), 4 * (t + 1) + 4)])
        # upfront: pair-0 stage 0 + its v tiles
        qk_chain(0, 0, 0)
        qk_chain(0, 1, 0)
        for tt in range(4):
            v_chain(tt)

        pend = []
        queue = []
        for (p, t, kk) in seq:
            blk = 4 * (t + 1)
            if kk == 0:
                queue = list(work_during.get((p, t), ()))
            attn_s_part(p, t, kk)
            # front-load filler: the CASTs must complete well before the
            # block boundary, where the previous normalize clogs the DVE
            for _ in range(min(2, len(queue))):
                queue.pop(0)()
            pend.append((p, t, kk))
            if len(pend) > 4:
                pp_, pj, pkk = pend.pop(0)
                attn_pv_part(pp_, pj, pkk)
        for pp_, pj, pkk in pend:
            attn_pv_part(pp_, pj, pkk)

    nc.compile()
    return nc


def _get_nc():
    global _NC
    if _NC is None:
        _NC = _build()
    return _NC


def _make_in_maps(x, W_attn):
    x = np.asarray(x, dtype=np.float32)
    W = np.asarray(W_attn, dtype=np.float32)
    wq, wk, wv = W[0:C], W[C:2 * C], W[2 * C:3 * C]
    in_maps = []
    for c in range(NCORES):
        b, g = c // 4, c % 4
        heads = [HPC * g + i for i in range(HPC)]
        xTb = np.ascontiguousarray(x[b].T).astype(NPBF16)
        qrows = np.concatenate([wq[D * h:D * h + D] for h in heads], axis=0)
        krows = np.concatenate([wk[D * h:D * h + D] for h in heads], axis=0)
        wqk_np = np.ascontiguousarray(
            np.concatenate([qrows, krows], 0).T).astype(NPBF16)
        wv_np = np.zeros((C, HPC * 65), np.float32)
        for i, h in enumerate(heads):
            wv_np[:, 65 * i:65 * i + D] = wv[D * h:D * h + D].T
        in_maps.append({"xT": xTb, "wqk": wqk_np, "wv": wv_np.astype(NPBF16),
                        "cm": _causal_masks()})
    return in_maps


def _causal_masks():
    r = np.arange(128)[:, None]
    return (np.arange(128)[None, :] >= r).astype(NPBF16)


def _execute(in_maps, trace=False):
    return run_bass_kernel_spmd(_get_nc(), in_maps,
                                core_ids=list(range(NCORES)), trace=trace)


def _assemble(results):
    y = np.empty((B, T, C), np.float32)
    for c in range(NCORES):
        b, g = c // 4, c % 4
        yc = results[c]["y"]
        for i in range(HPC):
            h = HPC * g + i
            y[b, :, D * h:D * h + D] = yc[i].T
    return y


def kernel(x, W_attn):
    res = _execute(_make_in_maps(x, W_attn), trace=False)
    return _assemble(res.results)


# revision 55
# speedup vs baseline: 1.0215x; 1.0128x over previous
"""Causal self-attention (B=2, T=2048, C=1024, H=16) on 8 TRN2 NeuronCores.

Sharding: core c handles batch b = c//4 and heads 4*(c%4) .. 4*(c%4)+3
(data-parallel over B, tensor-parallel over heads; full K/V for its heads
is computed locally from the core's QKV projection slice).

Per-core dataflow (all matmuls bf16 inputs, fp32 PSUM accumulation):
  - host passes xT = x[b].T [C,T], wqk = [Wq_h|Wk_h]^T [C,512],
    wv = [Wv_h0|0|...|Wv_h3|0]^T [C,260] (65-wide blocks, last col zero),
    and a [128,128] causal-mask tile, all bf16
  - qT [128,T] per pair via projection matmuls (contraction c on partitions)
  - kT per head is stored ZERO-PADDED to 128 partitions: kpe = [k_even; 0],
    kpo = [0; k_odd].  S matmuls then run with K=128 in the SAME 128x128 PE
    array configuration as every other matmul in the kernel -- one mode for
    the whole PE stream means no 64<->128 reconfiguration drains and
    back-to-back matmuls pipeline at N cycles.  (The pad rows multiply the
    other head's q rows by zero.)
  - v [t,260] with a ones column appended per head (65th of each block)
  - s [k=128, 1024] = [S_even|S_odd] per chunk; exp on ScalarE (scale=1/8
    fused) over only the causally-live columns; causal masking of the
    partial diagonal 128-col block is a DVE multiply with the precomputed
    mask (GpSimd's in-order queue is reserved for the normalize
    partition-broadcasts -- a gpsimd mask would queue behind exps and stall
    the normalize chain, which the next block's PV allocation waits on)
  - y^T [65, 512] += V'.T @ P^T accumulated over k-tiles; row 64 = softmax
    denominators (from the ones column)
  - normalize per q-block: the PSUM-freeing copies run first (so the next
    block's ye/yo never waits), then reciprocal_approx_fast + gpsimd
    partition_broadcast + multiply; DMA y^T[h] [64,512] out per block
  - host transposes/concats heads.

Schedule: one flat software pipeline over both pairs, block-interleaved
(p0,b0),(p1,b0),(p0,b1),... with PV lagging S by 4 chunks so ScalarE's exp
queue stays full; projection chains are front-loaded into the preceding
sub-block's first chunks (their CASTs must clear the DVE queue before the
block-boundary normalize lands on it).  PSUM: 2 double-bank s slots +
2 chain banks + ye/yo = 8 banks exactly.
"""

import os
import sys
import types
import numpy as np
import ml_dtypes

import concourse.bass as bass
import concourse.mybir as mybir
import concourse.tile as tile
from concourse import bacc
from concourse.bass_utils import run_bass_kernel_spmd

B, T, C, H = 2, 2048, 1024, 16
D = 64
NCORES = 8
HPC = 4          # heads per core
NQB = 4          # q blocks of 512
QB = 512
F32 = mybir.dt.float32
F32R = mybir.dt.float32r
BF16 = mybir.dt.bfloat16
NPBF16 = ml_dtypes.bfloat16
EXP = mybir.ActivationFunctionType.Exp
MULT = mybir.AluOpType.mult
IS_GE = mybir.AluOpType.is_ge


def _install_profhook():
    """Register the NTFF profile hook shim so BASS_TRACE=1 works; harmless
    no-op (graceful trace skip) when the axon .so lacks profiling."""
    if "antenv.axon_hooks" not in sys.modules:
        mod = types.ModuleType("antenv.axon_hooks")
        mod._hook = None
        mod.set_axon_ntff_profile_hook = lambda h: setattr(mod, "_hook", h)
        mod.get_axon_ntff_profile_hook = lambda: mod._hook
        sys.modules["antenv.axon_hooks"] = mod
        try:
            import antenv
            antenv.axon_hooks = mod
        except ImportError:
            pass
    try:
        from trn_agent_boot.trn_boot import _ntff_profile_via_ctypes
        sys.modules["antenv.axon_hooks"].set_axon_ntff_profile_hook(
            _ntff_profile_via_ctypes("/opt/axon/libaxon_pjrt.so")
        )
        import concourse.bass_utils as bu
        bu.upload_artifacts = lambda tmpdir: tmpdir
    except Exception:
        pass


_install_profhook()

_NC = None


def _build():
    nc = bacc.Bacc("TRN2", target_bir_lowering=False, debug=False,
                   num_devices=NCORES)
    xT_d = nc.declare_dram_parameter("xT", [C, T], BF16, isOutput=False)
    wqk_d = nc.declare_dram_parameter("wqk", [C, 2 * HPC * D], BF16,
                                      isOutput=False)
    wv_d = nc.declare_dram_parameter("wv", [C, HPC * 65], BF16,
                                     isOutput=False)
    cm_d = nc.declare_dram_parameter("cm", [128, 128], BF16, isOutput=False)
    y_d = nc.declare_dram_parameter("y", [HPC, D, T], F32, isOutput=True)

    from contextlib import ExitStack
    with tile.TileContext(nc) as tc, ExitStack() as ctx:
        sb = ctx.enter_context(tc.tile_pool(name="sb", bufs=1))
        pp = ctx.enter_context(tc.tile_pool(name="pp", bufs=10))
        yp = ctx.enter_context(tc.tile_pool(name="yp", bufs=4))
        # dedicated pools: chains double-buffered (2 banks) so S matmuls
        # never wait on a chain's CAST, s tiles 2x2 banks, ye/yo 2 banks
        psp = ctx.enter_context(tc.tile_pool(name="psp", bufs=2, space="PSUM"))
        ps = ctx.enter_context(tc.tile_pool(name="ps", bufs=2, space="PSUM"))
        psy = ctx.enter_context(tc.tile_pool(name="psy", bufs=1, space="PSUM"))

        # per-(c, t-block) x^T tiles; per-t-block q/k tiles -> fine-grain deps
        xTt = [[sb.tile([128, 512], BF16, name=f"xT{c}_{tb}")
                for tb in range(4)] for c in range(8)]
        wqks = [sb.tile([128, 512], BF16, name=f"wqk{i}") for i in range(8)]
        wvs = [sb.tile([128, 260], BF16, name=f"wv{i}") for i in range(8)]
        qs = [[sb.tile([128, 512], BF16, name=f"q{p}_{tb}") for tb in range(4)]
              for p in range(2)]
        # zero-padded per-head k tiles: kpe = [k_even; 0], kpo = [0; k_odd]
        kpe = [[sb.tile([128, 512], BF16, name=f"kpe{p}_{tb}")
                for tb in range(4)] for p in range(2)]
        kpo = [[sb.tile([128, 512], BF16, name=f"kpo{p}_{tb}")
                for tb in range(4)] for p in range(2)]
        vs = [sb.tile([128, 260], BF16, name=f"v_{t}") for t in range(16)]
        ones2 = sb.tile([128, 4], F32, name="ones2")

        # warm-up first: keep the PE's HAM activity monitor busy while the
        # input DMAs land, so real matmuls start at 2.4 GHz instead of 1.2
        wupf = sb.tile([128, 512], F32, name="wupf")
        nc.gpsimd.memset(wupf[:], 0.5)
        nc.gpsimd.memset(ones2[:], 1.0)
        wup = sb.tile([128, 512], F32R, name="wup")
        nc.vector.tensor_copy(wup[:], wupf[:])
        wups = psp.tile([128, 512], F32, name="wups", tag="pmm")
        for _ in range(8):
            nc.tensor.matmul(wups[:], wup[:, 0:128], wup[:], start=True,
                             stop=True)
        # zero the pad halves of the k tiles on the (idle-at-start) vector
        # engine
        for p in range(2):
            for tb in range(4):
                nc.vector.memset(kpe[p][tb][64:128, :], 0.0)
                nc.vector.memset(kpo[p][tb][0:64, :], 0.0)

        # host-provided causal-mask tile (keep iff col >= row); the per-chunk
        # mask is a cheap DVE multiply, keeping GpSimd's in-order queue free
        # for the normalize partition-broadcasts (a gpsimd mask queues behind
        # exps and would delay them).
        cmt = sb.tile([128, 128], BF16, name="cmt")
        nc.sync.dma_start(cmt[:], cm_d.ap()[:, :])

        # DMA order: (wqk[c], xT[c,0]) pairs so the first projection chain
        # can start after ~2 transfers; then wv, then xT t-blocks 1..3
        for c in range(8):
            cr = slice(c * 128, (c + 1) * 128)
            nc.sync.dma_start(wqks[c][:], wqk_d.ap()[cr, :])
            nc.sync.dma_start(xTt[c][0][:], xT_d.ap()[cr, 0:512])
        for c in range(8):
            nc.sync.dma_start(wvs[c][:], wv_d.ap()[c * 128:(c + 1) * 128, :])
        for tb in range(1, 4):
            for c in range(8):
                nc.sync.dma_start(
                    xTt[c][tb][:],
                    xT_d.ap()[c * 128:(c + 1) * 128, tb * 512:(tb + 1) * 512])

        def qk_chain(p, ft_kind, tb):
            """One projection chain: q (ft_kind=0) or k (ft_kind=1) of pair p,
            t-block tb.  q lands in qs; k is split into the two zero-padded
            per-head tiles (no partition shift: even head keeps rows 0:64,
            odd head keeps rows 64:128)."""
            ft = p if ft_kind == 0 else 2 + p
            mm = psp.tile([128, 512], F32, name=f"pqk{p}_{ft}_{tb}", tag="pmm")
            for c in range(8):
                nc.tensor.matmul(mm[:],
                                 wqks[c][:, ft * 128:(ft + 1) * 128],
                                 xTt[c][tb][:],
                                 start=(c == 0), stop=(c == 7))
            if ft_kind == 0:
                nc.vector.tensor_copy(qs[p][tb][:], mm[:])
            else:
                nc.vector.tensor_copy(kpe[p][tb][0:64, :], mm[0:64, :])
                nc.vector.tensor_copy(kpo[p][tb][64:128, :], mm[64:128, :])

        def v_chain(tt):
            """Combined v projection for one t-tile (all 4 heads, N=260)."""
            tb, sub = tt // 4, tt % 4
            mmv = psp.tile([128, 260], F32, name=f"pv{tt}", tag="pmm")
            for c in range(8):
                nc.tensor.matmul(mmv[:],
                                 xTt[c][tb][:, sub * 128:(sub + 1) * 128],
                                 wvs[c][:],
                                 start=(c == 0), stop=(c == 7))
            nc.vector.tensor_copy(vs[tt][:], mmv[:])
            nc.vector.tensor_copy(vs[tt][:, 64:260:65], ones2[:])

        ptiles = {}

        def attn_s_part(p, j, kk):
            """S matmuls + exp + causal mask for chunk (p, j, kk).

            Diagonal k-tiles only have valid scores for q >= k, i.e. local
            q >= off = 128*(kk-4j); the matmuls/exp/PV all start at column
            off, and the mask multiply zeroes the partial 128-col block's
            lower triangle."""
            off = max(0, 128 * (kk - 4 * j))
            roff = off
            s = ps.tile([128, 1024], F32, name=f"s{p}_{j}_{kk}", tag="s")
            ktb, ksub = kk // 4, (kk % 4) * 128
            nc.tensor.matmul(s[:, roff:512],
                             kpe[p][ktb][:, ksub:ksub + 128],
                             qs[p][j][:, roff:512],
                             start=True, stop=True)
            nc.tensor.matmul(s[:, 512 + roff:1024],
                             kpo[p][ktb][:, ksub:ksub + 128],
                             qs[p][j][:, roff:512],
                             start=True, stop=True)
            pt = pp.tile([128, 1024], BF16, name=f"pt{p}_{j}_{kk}", tag="pt")
            sv = s[:].rearrange("p (b q) -> p b q", b=2)[:, :, roff:512]
            pv = pt[:].rearrange("p (b q) -> p b q", b=2)[:, :, roff:512]
            nc.scalar.activation(pv, sv, EXP, scale=0.125)
            if kk >= 4 * j:
                # causal mask both head halves in one DVE multiply over just
                # the partial 128-col diagonal block at [off, off+128)
                v3 = pt[:].rearrange("p (b q) -> p b q", b=2)[:, :,
                                                             off:off + 128]
                m3 = cmt[:].rearrange("p (b q) -> p b q", b=1).to_broadcast(
                    (128, 2, 128))
                nc.vector.tensor_tensor(v3, v3, m3, op=MULT)
            ptiles[(p, j, kk)] = pt

        states = {}

        def normalize(p, j, state):
            """Both heads fused: the four PSUM-freeing copies run FIRST (so
            the next block's ye/yo allocation never waits on the rest of the
            chain), then one stacked recip/broadcast/multiply and one DMA."""
            ye, yo = state["ye"], state["yo"]
            # for the very last block ScalarE is idle: run the odd head's
            # copies there so both heads' normalize chains overlap in the
            # kernel's tail
            last_blk = (p, j) == (1, NQB - 1)
            ysbe = yp.tile([65, 512], F32, name=f"ysbe{p}_{j}", tag="ysbe")
            nc.vector.tensor_copy(ysbe[:], ye[:])
            ysbo = yp.tile([65, 512], F32, name=f"ysbo{p}_{j}", tag="ysbo")
            (nc.scalar.copy if last_blk else nc.vector.tensor_copy)(
                ysbo[:], yo[:])
            sse = yp.tile([1, 512], F32, name=f"sse{p}_{j}", tag="sse")
            nc.vector.tensor_copy(sse[:], ye[64:65, :])
            sso = yp.tile([1, 512], F32, name=f"sso{p}_{j}", tag="sso")
            (nc.scalar.copy if last_blk else nc.vector.tensor_copy)(
                sso[:], yo[64:65, :])
            rse = yp.tile([1, 512], F32, name=f"rse{p}_{j}", tag="rse")
            nc.vector.reciprocal_approx_fast(out=rse[:], in_=sse[:])
            rso = yp.tile([1, 512], F32, name=f"rso{p}_{j}", tag="rso")
            nc.vector.reciprocal_approx_fast(out=rso[:], in_=sso[:])
            bse = yp.tile([64, 512], F32, name=f"bse{p}_{j}", tag="bse")
            nc.gpsimd.partition_broadcast(bse[:], rse[:], channels=64)
            bso = yp.tile([64, 512], F32, name=f"bso{p}_{j}", tag="bso")
            nc.gpsimd.partition_broadcast(bso[:], rso[:], channels=64)
            yne = yp.tile([64, 512], F32, name=f"yne{p}_{j}", tag="yne")
            nc.vector.tensor_tensor(yne[:], ysbe[0:64, :], bse[:], op=MULT)
            nc.sync.dma_start(
                y_d.ap()[2 * p, :, j * 512:(j + 1) * 512], yne[:])
            yno = yp.tile([64, 512], F32, name=f"yno{p}_{j}", tag="yno")
            nc.vector.tensor_tensor(yno[:], ysbo[0:64, :], bso[:], op=MULT)
            nc.sync.dma_start(
                y_d.ap()[2 * p + 1, :, j * 512:(j + 1) * 512], yno[:])

        def attn_pv_part(p, j, kk):
            """PV accumulation for chunk (p, j, kk); finishes the q-block
            with normalize on its last k-tile."""
            state = states.setdefault((p, j), {})
            nkt = 4 * (j + 1)
            if kk == 0:
                state["ye"] = psy.tile([65, 512], F32,
                                       name=f"ye{p}_{j}", tag="ye")
                state["yo"] = psy.tile([65, 512], F32,
                                       name=f"yo{p}_{j}", tag="yo")
            pt = ptiles.pop((p, j, kk))
            first, last = (kk == 0), (kk == nkt - 1)
            # skip columns left of off (all-zero P above the causal
            # diagonal); their y contribution is zero and PSUM keeps the
            # prior partials there
            roff = 0 if first else max(0, 128 * (kk - 4 * j))
            nc.tensor.matmul(state["ye"][:, roff:512],
                             vs[kk][:, 130 * p:130 * p + 65],
                             pt[:, roff:512],
                             start=first, stop=last)
            nc.tensor.matmul(state["yo"][:, roff:512],
                             vs[kk][:, 130 * p + 65:130 * p + 130],
                             pt[:, 512 + roff:1024],
                             start=first, stop=last)
            if last:
                normalize(p, j, state)

        # One flat software pipeline over BOTH pairs, block-interleaved:
        # (p0,b0),(p1,b0),(p0,b1),(p1,b1),...  PV lags S by 3 chunks so
        # ScalarE's exp queue stays full; projection chains are spread across
        # the PRECEDING sub-block's chunks as PE filler (work for (p1,t)
        # during (p0,t); work for (p0,t+1) and its v tiles during (p1,t)).
        seq = [(p, t, kk) for t in range(NQB) for p in range(2)
               for kk in range(4 * (t + 1))]
        work_during = {}
        for t in range(NQB):
            work_during[(0, t)] = [lambda t=t: qk_chain(1, 0, t),
                                   lambda t=t: qk_chain(1, 1, t)]
            if t < NQB - 1:
                work_during[(1, t)] = (
                    [lambda t=t: qk_chain(0, 0, t + 1),
                     lambda t=t: qk_chain(0, 1, t + 1)] +
                    [lambda tt=tt: v_chain(tt)
                     for tt in range(4 * (t + 1), 4 * (t + 1) + 4)])
        # upfront: pair-0 stage 0 + its v tiles
        qk_chain(0, 0, 0)
        qk_chain(0, 1, 0)
        for tt in range(4):
            v_chain(tt)

        pend = []
        queue = []
        for (p, t, kk) in seq:
            blk = 4 * (t + 1)
            if kk == 0:
                queue = list(work_during.get((p, t), ()))
            attn_s_part(p, t, kk)
            # front-load filler: the CASTs must complete well before the
            # block boundary, where the previous normalize clogs the DVE
            for _ in range(min(2, len(queue))):
                queue.pop(0)()
            pend.append((p, t, kk))
            if len(pend) > 4:
                pp_, pj, pkk = pend.pop(0)
                attn_pv_part(pp_, pj, pkk)
        for pp_, pj, pkk in pend:
            attn_pv_part(pp_, pj, pkk)

    nc.compile()
    return nc


def _get_nc():
    global _NC
    if _NC is None:
        _NC = _build()
    return _NC


def _make_in_maps(x, W_attn):
    x = np.asarray(x, dtype=np.float32)
    W = np.asarray(W_attn, dtype=np.float32)
    wq, wk, wv = W[0:C], W[C:2 * C], W[2 * C:3 * C]
    in_maps = []
    for c in range(NCORES):
        b, g = c // 4, c % 4
        heads = [HPC * g + i for i in range(HPC)]
        xTb = np.ascontiguousarray(x[b].T).astype(NPBF16)
        qrows = np.concatenate([wq[D * h:D * h + D] for h in heads], axis=0)
        krows = np.concatenate([wk[D * h:D * h + D] for h in heads], axis=0)
        wqk_np = np.ascontiguousarray(
            np.concatenate([qrows, krows], 0).T).astype(NPBF16)
        wv_np = np.zeros((C, HPC * 65), np.float32)
        for i, h in enumerate(heads):
            wv_np[:, 65 * i:65 * i + D] = wv[D * h:D * h + D].T
        in_maps.append({"xT": xTb, "wqk": wqk_np, "wv": wv_np.astype(NPBF16),
                        "cm": _causal_masks()})
    return in_maps


def _causal_masks():
    r = np.arange(128)[:, None]
    return (np.arange(128)[None, :] >= r).astype(NPBF16)


def _execute(in_maps, trace=False):
    return run_bass_kernel_spmd(_get_nc(), in_maps,
                                core_ids=list(range(NCORES)), trace=trace)


def _assemble(results):
    y = np.empty((B, T, C), np.float32)
    for c in range(NCORES):
        b, g = c // 4, c % 4
        yc = results[c]["y"]
        for i in range(HPC):
            h = HPC * g + i
            y[b, :, D * h:D * h + D] = yc[i].T
    return y


def kernel(x, W_attn):
    res = _execute(_make_in_maps(x, W_attn), trace=False)
    return _assemble(res.results)


# revision 56
# speedup vs baseline: 1.0255x; 1.0039x over previous
"""Causal self-attention (B=2, T=2048, C=1024, H=16) on 8 TRN2 NeuronCores.

Sharding: core c handles batch b = c//4 and heads 4*(c%4) .. 4*(c%4)+3
(data-parallel over B, tensor-parallel over heads; full K/V for its heads
is computed locally from the core's QKV projection slice).

Per-core dataflow (all matmuls bf16 inputs, fp32 PSUM accumulation):
  - host passes xT = x[b].T [C,T], wqk = [Wq_h|Wk_h]^T [C,512],
    wv = [Wv_h0|0|...|Wv_h3|0]^T [C,260] (65-wide blocks, last col zero),
    and a [128,128] causal-mask tile, all bf16
  - qT [128,T] per pair via projection matmuls (contraction c on partitions)
  - kT per head is stored ZERO-PADDED to 128 partitions: kpe = [k_even; 0],
    kpo = [0; k_odd].  S matmuls then run with K=128 in the SAME 128x128 PE
    array configuration as every other matmul in the kernel -- one mode for
    the whole PE stream means no 64<->128 reconfiguration drains and
    back-to-back matmuls pipeline at N cycles.  (The pad rows multiply the
    other head's q rows by zero.)
  - v [t,260] with a ones column appended per head (65th of each block)
  - s [k=128, 1024] = [S_even|S_odd] per chunk; exp on ScalarE (scale=1/8
    fused) over only the causally-live columns; causal masking of the
    partial diagonal 128-col block is a DVE multiply with the precomputed
    mask (GpSimd's in-order queue is reserved for the normalize
    partition-broadcasts -- a gpsimd mask would queue behind exps and stall
    the normalize chain, which the next block's PV allocation waits on)
  - y^T [65, 512] += V'.T @ P^T accumulated over k-tiles; row 64 = softmax
    denominators (from the ones column)
  - normalize per q-block: the PSUM-freeing copies run first (so the next
    block's ye/yo never waits), then reciprocal_approx_fast + gpsimd
    partition_broadcast + multiply; DMA y^T[h] [64,512] out per block
  - host transposes/concats heads.

Schedule: one flat software pipeline over both pairs, block-interleaved
(p0,b0),(p1,b0),(p0,b1),... with PV lagging S by 4 chunks so ScalarE's exp
queue stays full; projection chains are front-loaded into the preceding
sub-block's first chunks (their CASTs must clear the DVE queue before the
block-boundary normalize lands on it).  PSUM: 2 double-bank s slots +
2 chain banks + ye/yo = 8 banks exactly.
"""

import os
import sys
import types
import numpy as np
import ml_dtypes

import concourse.bass as bass
import concourse.mybir as mybir
import concourse.tile as tile
from concourse import bacc
from concourse.bass_utils import run_bass_kernel_spmd

B, T, C, H = 2, 2048, 1024, 16
D = 64
NCORES = 8
HPC = 4          # heads per core
NQB = 4          # q blocks of 512
QB = 512
F32 = mybir.dt.float32
F32R = mybir.dt.float32r
BF16 = mybir.dt.bfloat16
NPBF16 = ml_dtypes.bfloat16
EXP = mybir.ActivationFunctionType.Exp
MULT = mybir.AluOpType.mult
IS_GE = mybir.AluOpType.is_ge


def _install_profhook():
    """Register the NTFF profile hook shim so BASS_TRACE=1 works; harmless
    no-op (graceful trace skip) when the axon .so lacks profiling."""
    if "antenv.axon_hooks" not in sys.modules:
        mod = types.ModuleType("antenv.axon_hooks")
        mod._hook = None
        mod.set_axon_ntff_profile_hook = lambda h: setattr(mod, "_hook", h)
        mod.get_axon_ntff_profile_hook = lambda: mod._hook
        sys.modules["antenv.axon_hooks"] = mod
        try:
            import antenv
            antenv.axon_hooks = mod
        except ImportError:
            pass
    try:
        from trn_agent_boot.trn_boot import _ntff_profile_via_ctypes
        sys.modules["antenv.axon_hooks"].set_axon_ntff_profile_hook(
            _ntff_profile_via_ctypes("/opt/axon/libaxon_pjrt.so")
        )
        import concourse.bass_utils as bu
        bu.upload_artifacts = lambda tmpdir: tmpdir
    except Exception:
        pass


_install_profhook()

_NC = None


def _build():
    nc = bacc.Bacc("TRN2", target_bir_lowering=False, debug=False,
                   num_devices=NCORES)
    xT_d = nc.declare_dram_parameter("xT", [C, T], BF16, isOutput=False)
    wqk_d = nc.declare_dram_parameter("wqk", [C, 2 * HPC * D], BF16,
                                      isOutput=False)
    wv_d = nc.declare_dram_parameter("wv", [C, HPC * 65], BF16,
                                     isOutput=False)
    cm_d = nc.declare_dram_parameter("cm", [128, 128], BF16, isOutput=False)
    y_d = nc.declare_dram_parameter("y", [HPC, D, T], F32, isOutput=True)

    from contextlib import ExitStack
    with tile.TileContext(nc) as tc, ExitStack() as ctx:
        sb = ctx.enter_context(tc.tile_pool(name="sb", bufs=1))
        pp = ctx.enter_context(tc.tile_pool(name="pp", bufs=10))
        yp = ctx.enter_context(tc.tile_pool(name="yp", bufs=4))
        # dedicated pools: chains double-buffered (2 banks) so S matmuls
        # never wait on a chain's CAST, s tiles 2x2 banks, ye/yo 2 banks
        psp = ctx.enter_context(tc.tile_pool(name="psp", bufs=2, space="PSUM"))
        ps = ctx.enter_context(tc.tile_pool(name="ps", bufs=2, space="PSUM"))
        psy = ctx.enter_context(tc.tile_pool(name="psy", bufs=1, space="PSUM"))

        # per-(c, t-block) x^T tiles; per-t-block q/k tiles -> fine-grain deps
        xTt = [[sb.tile([128, 512], BF16, name=f"xT{c}_{tb}")
                for tb in range(4)] for c in range(8)]
        wqks = [sb.tile([128, 512], BF16, name=f"wqk{i}") for i in range(8)]
        wvs = [sb.tile([128, 260], BF16, name=f"wv{i}") for i in range(8)]
        qs = [[sb.tile([128, 512], BF16, name=f"q{p}_{tb}") for tb in range(4)]
              for p in range(2)]
        # zero-padded per-head k tiles: kpe = [k_even; 0], kpo = [0; k_odd]
        kpe = [[sb.tile([128, 512], BF16, name=f"kpe{p}_{tb}")
                for tb in range(4)] for p in range(2)]
        kpo = [[sb.tile([128, 512], BF16, name=f"kpo{p}_{tb}")
                for tb in range(4)] for p in range(2)]
        vs = [sb.tile([128, 260], BF16, name=f"v_{t}") for t in range(16)]
        ones2 = sb.tile([128, 4], F32, name="ones2")

        nc.gpsimd.memset(ones2[:], 1.0)
        # no PE warmup: the first projection chains are DMA-paced anyway and
        # warm the HAM with real work instead of dummy matmuls
        # zero the pad halves of the k tiles on the (idle-at-start) vector
        # engine
        for p in range(2):
            for tb in range(4):
                nc.vector.memset(kpe[p][tb][64:128, :], 0.0)
                nc.vector.memset(kpo[p][tb][0:64, :], 0.0)

        # host-provided causal-mask tile (keep iff col >= row); the per-chunk
        # mask is a cheap DVE multiply, keeping GpSimd's in-order queue free
        # for the normalize partition-broadcasts (a gpsimd mask queues behind
        # exps and would delay them).
        cmt = sb.tile([128, 128], BF16, name="cmt")
        nc.sync.dma_start(cmt[:], cm_d.ap()[:, :])

        # DMA order: (wqk[c], xT[c,0]) pairs so the first projection chain
        # can start after ~2 transfers; then wv, then xT t-blocks 1..3
        for c in range(8):
            cr = slice(c * 128, (c + 1) * 128)
            nc.sync.dma_start(wqks[c][:], wqk_d.ap()[cr, :])
            nc.sync.dma_start(xTt[c][0][:], xT_d.ap()[cr, 0:512])
        for c in range(8):
            nc.sync.dma_start(wvs[c][:], wv_d.ap()[c * 128:(c + 1) * 128, :])
        for tb in range(1, 4):
            for c in range(8):
                nc.sync.dma_start(
                    xTt[c][tb][:],
                    xT_d.ap()[c * 128:(c + 1) * 128, tb * 512:(tb + 1) * 512])

        def qk_chain(p, ft_kind, tb):
            """One projection chain: q (ft_kind=0) or k (ft_kind=1) of pair p,
            t-block tb.  q lands in qs; k is split into the two zero-padded
            per-head tiles (no partition shift: even head keeps rows 0:64,
            odd head keeps rows 64:128)."""
            ft = p if ft_kind == 0 else 2 + p
            mm = psp.tile([128, 512], F32, name=f"pqk{p}_{ft}_{tb}", tag="pmm")
            for c in range(8):
                nc.tensor.matmul(mm[:],
                                 wqks[c][:, ft * 128:(ft + 1) * 128],
                                 xTt[c][tb][:],
                                 start=(c == 0), stop=(c == 7))
            if ft_kind == 0:
                nc.vector.tensor_copy(qs[p][tb][:], mm[:])
            else:
                nc.vector.tensor_copy(kpe[p][tb][0:64, :], mm[0:64, :])
                nc.vector.tensor_copy(kpo[p][tb][64:128, :], mm[64:128, :])

        def v_chain(tt):
            """Combined v projection for one t-tile (all 4 heads, N=260)."""
            tb, sub = tt // 4, tt % 4
            mmv = psp.tile([128, 260], F32, name=f"pv{tt}", tag="pmm")
            for c in range(8):
                nc.tensor.matmul(mmv[:],
                                 xTt[c][tb][:, sub * 128:(sub + 1) * 128],
                                 wvs[c][:],
                                 start=(c == 0), stop=(c == 7))
            nc.vector.tensor_copy(vs[tt][:], mmv[:])
            nc.vector.tensor_copy(vs[tt][:, 64:260:65], ones2[:])

        ptiles = {}

        def attn_s_part(p, j, kk):
            """S matmuls + exp + causal mask for chunk (p, j, kk).

            Diagonal k-tiles only have valid scores for q >= k, i.e. local
            q >= off = 128*(kk-4j); the matmuls/exp/PV all start at column
            off, and the mask multiply zeroes the partial 128-col block's
            lower triangle."""
            off = max(0, 128 * (kk - 4 * j))
            roff = off
            s = ps.tile([128, 1024], F32, name=f"s{p}_{j}_{kk}", tag="s")
            ktb, ksub = kk // 4, (kk % 4) * 128
            nc.tensor.matmul(s[:, roff:512],
                             kpe[p][ktb][:, ksub:ksub + 128],
                             qs[p][j][:, roff:512],
                             start=True, stop=True)
            nc.tensor.matmul(s[:, 512 + roff:1024],
                             kpo[p][ktb][:, ksub:ksub + 128],
                             qs[p][j][:, roff:512],
                             start=True, stop=True)
            pt = pp.tile([128, 1024], BF16, name=f"pt{p}_{j}_{kk}", tag="pt")
            sv = s[:].rearrange("p (b q) -> p b q", b=2)[:, :, roff:512]
            pv = pt[:].rearrange("p (b q) -> p b q", b=2)[:, :, roff:512]
            nc.scalar.activation(pv, sv, EXP, scale=0.125)
            if kk >= 4 * j:
                # causal mask both head halves in one DVE multiply over just
                # the partial 128-col diagonal block at [off, off+128)
                v3 = pt[:].rearrange("p (b q) -> p b q", b=2)[:, :,
                                                             off:off + 128]
                m3 = cmt[:].rearrange("p (b q) -> p b q", b=1).to_broadcast(
                    (128, 2, 128))
                nc.vector.tensor_tensor(v3, v3, m3, op=MULT)
            ptiles[(p, j, kk)] = pt

        states = {}

        def normalize(p, j, state):
            """Both heads fused: the four PSUM-freeing copies run FIRST (so
            the next block's ye/yo allocation never waits on the rest of the
            chain), then one stacked recip/broadcast/multiply and one DMA."""
            ye, yo = state["ye"], state["yo"]
            # for the very last block ScalarE is idle: run the odd head's
            # copies there so both heads' normalize chains overlap in the
            # kernel's tail
            last_blk = (p, j) == (1, NQB - 1)
            ysbe = yp.tile([65, 512], F32, name=f"ysbe{p}_{j}", tag="ysbe")
            nc.vector.tensor_copy(ysbe[:], ye[:])
            ysbo = yp.tile([65, 512], F32, name=f"ysbo{p}_{j}", tag="ysbo")
            (nc.scalar.copy if last_blk else nc.vector.tensor_copy)(
                ysbo[:], yo[:])
            sse = yp.tile([1, 512], F32, name=f"sse{p}_{j}", tag="sse")
            nc.vector.tensor_copy(sse[:], ye[64:65, :])
            sso = yp.tile([1, 512], F32, name=f"sso{p}_{j}", tag="sso")
            (nc.scalar.copy if last_blk else nc.vector.tensor_copy)(
                sso[:], yo[64:65, :])
            rse = yp.tile([1, 512], F32, name=f"rse{p}_{j}", tag="rse")
            nc.vector.reciprocal_approx_fast(out=rse[:], in_=sse[:])
            rso = yp.tile([1, 512], F32, name=f"rso{p}_{j}", tag="rso")
            nc.vector.reciprocal_approx_fast(out=rso[:], in_=sso[:])
            bse = yp.tile([64, 512], F32, name=f"bse{p}_{j}", tag="bse")
            nc.gpsimd.partition_broadcast(bse[:], rse[:], channels=64)
            bso = yp.tile([64, 512], F32, name=f"bso{p}_{j}", tag="bso")
            nc.gpsimd.partition_broadcast(bso[:], rso[:], channels=64)
            yne = yp.tile([64, 512], F32, name=f"yne{p}_{j}", tag="yne")
            nc.vector.tensor_tensor(yne[:], ysbe[0:64, :], bse[:], op=MULT)
            nc.sync.dma_start(
                y_d.ap()[2 * p, :, j * 512:(j + 1) * 512], yne[:])
            yno = yp.tile([64, 512], F32, name=f"yno{p}_{j}", tag="yno")
            nc.vector.tensor_tensor(yno[:], ysbo[0:64, :], bso[:], op=MULT)
            nc.sync.dma_start(
                y_d.ap()[2 * p + 1, :, j * 512:(j + 1) * 512], yno[:])

        def attn_pv_part(p, j, kk):
            """PV accumulation for chunk (p, j, kk); finishes the q-block
            with normalize on its last k-tile."""
            state = states.setdefault((p, j), {})
            nkt = 4 * (j + 1)
            if kk == 0:
                state["ye"] = psy.tile([65, 512], F32,
                                       name=f"ye{p}_{j}", tag="ye")
                state["yo"] = psy.tile([65, 512], F32,
                                       name=f"yo{p}_{j}", tag="yo")
            pt = ptiles.pop((p, j, kk))
            first, last = (kk == 0), (kk == nkt - 1)
            # skip columns left of off (all-zero P above the causal
            # diagonal); their y contribution is zero and PSUM keeps the
            # prior partials there
            roff = 0 if first else max(0, 128 * (kk - 4 * j))
            nc.tensor.matmul(state["ye"][:, roff:512],
                             vs[kk][:, 130 * p:130 * p + 65],
                             pt[:, roff:512],
                             start=first, stop=last)
            nc.tensor.matmul(state["yo"][:, roff:512],
                             vs[kk][:, 130 * p + 65:130 * p + 130],
                             pt[:, 512 + roff:1024],
                             start=first, stop=last)
            if last:
                normalize(p, j, state)

        # One flat software pipeline over BOTH pairs, block-interleaved:
        # (p0,b0),(p1,b0),(p0,b1),(p1,b1),...  PV lags S by 3 chunks so
        # ScalarE's exp queue stays full; projection chains are spread across
        # the PRECEDING sub-block's chunks as PE filler (work for (p1,t)
        # during (p0,t); work for (p0,t+1) and its v tiles during (p1,t)).
        seq = [(p, t, kk) for t in range(NQB) for p in range(2)
               for kk in range(4 * (t + 1))]
        work_during = {}
        for t in range(NQB):
            work_during[(0, t)] = [lambda t=t: qk_chain(1, 0, t),
                                   lambda t=t: qk_chain(1, 1, t)]
            if t < NQB - 1:
                work_during[(1, t)] = (
                    [lambda t=t: qk_chain(0, 0, t + 1),
                     lambda t=t: qk_chain(0, 1, t + 1)] +
                    [lambda tt=tt: v_chain(tt)
                     for tt in range(4 * (t + 1), 4 * (t + 1) + 4)])
        # upfront: pair-0 stage 0 + its v tiles
        qk_chain(0, 0, 0)
        qk_chain(0, 1, 0)
        for tt in range(4):
            v_chain(tt)

        pend = []
        queue = []
        for (p, t, kk) in seq:
            blk = 4 * (t + 1)
            if kk == 0:
                queue = list(work_during.get((p, t), ()))
            attn_s_part(p, t, kk)
            # front-load filler: the CASTs must complete well before the
            # block boundary, where the previous normalize clogs the DVE
            for _ in range(min(2, len(queue))):
                queue.pop(0)()
            pend.append((p, t, kk))
            if len(pend) > 4:
                pp_, pj, pkk = pend.pop(0)
                attn_pv_part(pp_, pj, pkk)
        for pp_, pj, pkk in pend:
            attn_pv_part(pp_, pj, pkk)

    nc.compile()
    return nc


def _get_nc():
    global _NC
    if _NC is None:
        _NC = _build()
    return _NC


def _make_in_maps(x, W_attn):
    x = np.asarray(x, dtype=np.float32)
    W = np.asarray(W_attn, dtype=np.float32)
    wq, wk, wv = W[0:C], W[C:2 * C], W[2 * C:3 * C]
    in_maps = []
    for c in range(NCORES):
        b, g = c // 4, c % 4
        heads = [HPC * g + i for i in range(HPC)]
        xTb = np.ascontiguousarray(x[b].T).astype(NPBF16)
        qrows = np.concatenate([wq[D * h:D * h + D] for h in heads], axis=0)
        krows = np.concatenate([wk[D * h:D * h + D] for h in heads], axis=0)
        wqk_np = np.ascontiguousarray(
            np.concatenate([qrows, krows], 0).T).astype(NPBF16)
        wv_np = np.zeros((C, HPC * 65), np.float32)
        for i, h in enumerate(heads):
            wv_np[:, 65 * i:65 * i + D] = wv[D * h:D * h + D].T
        in_maps.append({"xT": xTb, "wqk": wqk_np, "wv": wv_np.astype(NPBF16),
                        "cm": _causal_masks()})
    return in_maps


def _causal_masks():
    r = np.arange(128)[:, None]
    return (np.arange(128)[None, :] >= r).astype(NPBF16)


def _execute(in_maps, trace=False):
    return run_bass_kernel_spmd(_get_nc(), in_maps,
                                core_ids=list(range(NCORES)), trace=trace)


def _assemble(results):
    y = np.empty((B, T, C), np.float32)
    for c in range(NCORES):
        b, g = c // 4, c % 4
        yc = results[c]["y"]
        for i in range(HPC):
            h = HPC * g + i
            y[b, :, D * h:D * h + D] = yc[i].T
    return y


def kernel(x, W_attn):
    res = _execute(_make_in_maps(x, W_attn), trace=False)
    return _assemble(res.results)
